# revision 3
# baseline (speedup 1.0000x reference)
"""Distributed Trainium2 kernel for nn_AdaptiveAvgPoolSequence.

Computation (reference): bucketize N=65536 points into an 8x8 spatial grid,
take the per-bin mean of values [B, N, C] over the point axis, flatten to
[B, 64*C], then a Linear to [B, 512].

Sharding across 8 NeuronCores — bin-sharded, collective-free:
  - the host bucketizes coords (bit-exact vs the reference searchsorted),
    stable-sorts the point axis by bin id, zero-pads each bin to a multiple
    of 4 points (+~0.2%), and hands each core a contiguous run of NS'=8320
    sorted points.  Per-core outputs [B, 512] sum on the host (the Linear
    is linear; bias is added there too)
  - values and W are cast to bf16 ON THE HOST (numerically identical to an
    on-device cast), halving HBM traffic; value units stream as plain bf16
    copies alternating between the two HWDGE rings (sync=SP, scalar=ACT);
    W rides the otherwise-idle SWDGE ring
  - quad folding: the host arranges each unit so partition p holds 4
    consecutive same-bin points (bins are padded to multiples of 4, so
    every aligned quad of the sorted stream is same-bin).  The idle Vector
    engine folds the unit's 4 chunks into one [128, B*C] tile (3 bf16
    adds), and the PE runs the one-hot segment-sum matmul once per UNIT
    instead of once per chunk — a 4x cut of TensorE work, which profiling
    showed to be co-critical with the DMA stream
  - early-Linear overlap: sorted order means early bin slots stop receiving
    contributions partway through the stream.  The PSUM accumulation splits
    at unit FREEZE_FC (the host verifies slots < G0 are complete by then);
    the first G0 slots' transpose + Linear run under the last value units,
    leaving only L-G0 slots' tail work after the final DMA
"""

import numpy as np
import ml_dtypes

import concourse.bacc as bacc
import concourse.mybir as mybir
import concourse.tile as tile
from concourse.bass_utils import run_bass_kernel_spmd

BF16 = ml_dtypes.bfloat16

N_CORES = 8
B, N, C = 4, 65536, 256
J = 65                     # chunks of 128 points per core (8320 points)
NS = J * 128               # padded points per core
NTOT = N_CORES * NS        # padded stream length (66560)
FOLD = 4
FC = 17                    # folded units: 16 x 4 chunks + 1 raw chunk
HW = 64                    # 8x8 bins
L = 11                     # local bin-slot capacity per core
KK = L * C // 128          # 22 K-chunks of the per-core Linear contraction
OUT = 512
BC = B * C                 # 1024
G0 = 6                     # slots frozen (complete) by unit FREEZE_FC
FREEZE_FC = 13             # unit where the psum accumulation splits
WARMUP = 6                 # dummy matmuls to lift the PE clock early

# Bin edges Tx[1..8] == Ty[1..8] of jnp.linspace(-1-1e-6, 1+1e-6, 9) in
# float32, hardcoded as bit patterns so host comparisons match the
# reference searchsorted bit-for-bit.
_EDGE_BITS = np.array(
    [3208642572, 3204448264, 3196059656, 0,
     1048576008, 1056964616, 1061158924, 1065353224],
    dtype=np.uint32,
)
EDGES = _EDGE_BITS.view(np.float32)

_NCS = {}


def _build(early=True):
    f32 = mybir.dt.float32
    bf16 = mybir.dt.bfloat16
    is_eq = mybir.AluOpType.is_equal
    add = mybir.AluOpType.add
    LT = L - G0 if early else L     # slots handled in the tail

    nc = bacc.Bacc("TRN2", debug=False, num_devices=N_CORES)
    values = nc.dram_tensor("values", [128, J * B * C], bf16, kind="ExternalInput")
    binst_ext = nc.dram_tensor("binst", [128, FC], f32, kind="ExternalInput")
    rdiag_ext = nc.dram_tensor("recdiag", [L, L], bf16, kind="ExternalInput")
    # host pre-transposed: W[p, kk, o] = W_local[kk*128 + p, o]
    w_ext = nc.dram_tensor("W", [128, KK * OUT], bf16, kind="ExternalInput")
    out_ext = nc.dram_tensor("out", [B, OUT], f32, kind="ExternalOutput")

    with tile.TileContext(nc) as tc:
        with (
            tc.tile_pool(name="const", bufs=1) as cp,
            tc.tile_pool(name="vbp", bufs=6) as vbp,
            tc.tile_pool(name="vfp", bufs=4) as vfp,
            tc.tile_pool(name="sb", bufs=1) as sb,
            tc.tile_pool(name="pp", bufs=1, space="PSUM") as pp,
            tc.tile_pool(name="ppt", bufs=2, space="PSUM") as ppt,
            tc.tile_pool(name="pw", bufs=1, space="PSUM") as pw,
        ):
            vre = values.ap().rearrange("p (j z) -> p j z", j=J)
            w_bf = cp.tile([128, KK * OUT], bf16)
            wre = w_ext.ap().rearrange("p (kk o) -> p kk o", kk=KK)

            # binst leads the sync FIFO: the one-hots need it first
            binst = cp.tile([128, FC], f32)
            nc.sync.dma_start(binst[:], binst_ext.ap())

            def value_dma(f):
                # unit f covers chunks [4f, 4f+qd); plain bf16 copies
                # alternating between the two HWDGE rings
                qd = 4 if f < FC - 1 else 1
                vb = vbp.tile([128, 4 * BC], bf16)
                eng = nc.sync if f % 2 == 0 else nc.scalar
                eng.dma_start(
                    vb[:, 0:qd * BC].rearrange("p (j z) -> p j z", j=qd),
                    vre[:, 4 * f:4 * f + qd, :])
                return vb

            # prefetch the first four value units before any small setup
            vbs = {f: value_dma(f) for f in range(4)}
            # W on the otherwise-idle SWDGE ring: packet round-robin spreads
            # its 2.9 MB across the early stream without delaying any unit
            nc.gpsimd.dma_start(
                w_bf[:].rearrange("p (kk o) -> p kk o", kk=KK), wre[:])

            # PE warm-up: the clock ramps only under sustained matmul
            # activity; burn a short train on junk while the first units fly
            wu = cp.tile([128, OUT], bf16)
            nc.vector.memset(wu[:], 0.0)
            pjunk = pw.tile([128, OUT], f32)
            for _ in range(WARMUP):
                nc.tensor.matmul(pjunk[:], wu[:, 0:128], wu[:],
                                 start=True, stop=True)

            iotaL = cp.tile([128, L], f32)
            nc.gpsimd.iota(iotaL[:], pattern=[[1, L]], base=0,
                           channel_multiplier=0, allow_small_or_imprecise_dtypes=True)
            rdiag = cp.tile([L, L], bf16)
            nc.gpsimd.dma_start(rdiag[:], rdiag_ext.ap())

            # one-hots for all units: oh_all[p, h, f] = (iota[h] == binst[p, f])
            oh_all = sb.tile([128, L, FC], bf16)
            nc.vector.tensor_tensor(
                oh_all[:],
                iotaL[:].unsqueeze(2).broadcast_to([128, L, FC]),
                binst[:].unsqueeze(1).broadcast_to([128, L, FC]),
                is_eq)
            if early:
                # slot-(h+G0) one-hots at partition-base-0 slot index h, for
                # the post-FREEZE accumulators (matmul operands must sit at
                # partition base 0/32/64, so slots >= G0 get their own tiles)
                LT_ = L - G0
                iotaG = cp.tile([128, LT_], f32)
                nc.gpsimd.iota(iotaG[:], pattern=[[1, LT_]], base=G0,
                               channel_multiplier=0,
                               allow_small_or_imprecise_dtypes=True)
                oh_late = sb.tile([128, LT_, FC - FREEZE_FC], bf16)
                nc.vector.tensor_tensor(
                    oh_late[:],
                    iotaG[:].unsqueeze(2).broadcast_to(
                        [128, LT_, FC - FREEZE_FC]),
                    binst[:, FREEZE_FC:FC].unsqueeze(1).broadcast_to(
                        [128, LT_, FC - FREEZE_FC]),
                    is_eq)
                rdiagL = cp.tile([L - G0, L - G0], bf16)
                nc.sync.dma_start(rdiagL[:], rdiag[G0:L, G0:L])

            psum_a = pp.tile([L, 512], f32, tag="pa")
            psum_b = pp.tile([L, 512], f32, tag="pb")
            psum_o = pp.tile([B, OUT], f32, tag="po")
            lhst = [sb.tile([128, L * B], bf16, tag=f"lh{ch}", name=f"lhst{ch}")
                    for ch in range(2)]
            w_bf3 = w_bf[:].rearrange("p (kk o) -> p kk o", kk=KK)
            first_o = [True]

            def transpose_slots(s0, s1, src_bf, diag_ap):
                # pt[c, h-s0] = src[h-s0, b4*C + ch*128 + c] * recip[h]
                # (slot h lives on partition h-s0 of src_bf and diag_ap)
                for ch in range(2):
                    for b4 in range(B):
                        pt = ppt.tile([128, s1 - s0], f32)
                        lo = b4 * C + ch * 128
                        nc.tensor.matmul(pt[:], src_bf[0:s1 - s0, lo:lo + 128],
                                         diag_ap, start=True, stop=True)
                        dst = lhst[ch][:].rearrange(
                            "p (h q) -> p h q", q=B)[:, s0:s1, b4]
                        nc.any.tensor_copy(dst, pt[:])

            def linear_slots(s0, s1, last=False):
                for ch in range(2):
                    for h in range(s0, s1):
                        kk = h * 2 + ch
                        sp = last and ch == 1 and h == s1 - 1
                        nc.tensor.matmul(psum_o[:], lhst[ch][:, h * B:(h + 1) * B],
                                         w_bf3[:, kk, :],
                                         start=first_o[0], stop=sp)
                        first_o[0] = False

            # ---- value stream: fold 4 chunks on DVE, one-hot matmul per unit
            pa, pb = psum_a, psum_b
            for f in range(FC):
                vb = vbs.pop(f) if f in vbs else value_dma(f)
                if f < FC - 1:
                    vf = vfp.tile([128, BC], bf16)
                    nc.vector.tensor_tensor(
                        vf[:], vb[:, 0:BC], vb[:, BC:2 * BC], add)
                    nc.vector.tensor_tensor(
                        vf[:], vf[:], vb[:, 2 * BC:3 * BC], add)
                    nc.vector.tensor_tensor(
                        vf[:], vf[:], vb[:, 3 * BC:4 * BC], add)
                else:
                    vf = vb          # raw single chunk, no fold
                late = early and f >= FREEZE_FC
                oh = oh_late[:, :, f - FREEZE_FC] if late else oh_all[:, :, f]
                st = f == 0 or (early and f == FREEZE_FC)
                sp = f == FC - 1 or (early and f == FREEZE_FC - 1)
                nc.tensor.matmul(pa[:], oh, vf[:, 0:512], start=st, stop=sp)
                nc.tensor.matmul(pb[:], oh, vf[:, 512:1024], start=st, stop=sp)
                if early and f == FREEZE_FC - 1:
                    # slots < G0 are complete: save the frozen sums, then
                    # run their transpose+Linear under the remaining units
                    sumsA = sb.tile([L, BC], f32)
                    nc.vector.tensor_copy(sumsA[:, 0:512], psum_a[:])
                    nc.vector.tensor_copy(sumsA[:, 512:1024], psum_b[:])
                    sumsA_bf = sb.tile([G0, BC], bf16, name="sumsA_bf")
                    nc.vector.tensor_copy(sumsA_bf[:], sumsA[0:G0, :])
                    # shift the frozen rows of slots >= G0 to partition base
                    # 0 (SBUF->SBUF DMA moves across partitions); SWDGE ring
                    # is idle by now (W long landed), so it lands promptly
                    sumsAL = sb.tile([LT, BC], f32, name="sumsAL")
                    nc.gpsimd.dma_start(sumsAL[:], sumsA[G0:L, :])
                    transpose_slots(0, G0, sumsA_bf, rdiag[0:G0, 0:G0])
                    linear_slots(0, G0)
                    pa = pp.tile([LT, 512], f32, tag="pa2")
                    pb = pp.tile([LT, 512], f32, tag="pb2")

            # ---- tail: remaining slots' transpose + Linear ----
            s0 = L - LT
            sumsL_bf = sb.tile([LT, BC], bf16, name="sumsL_bf")
            if early:
                # slot s0+h accumulated on partition h post-FREEZE; add the
                # frozen pre-FREEZE partial sums
                nc.vector.tensor_tensor(
                    sumsL_bf[:, 0:512], pa[:], sumsAL[:, 0:512], add)
                nc.vector.tensor_tensor(
                    sumsL_bf[:, 512:1024], pb[:], sumsAL[:, 512:1024], add)
                transpose_slots(s0, L, sumsL_bf, rdiagL[:])
            else:
                nc.vector.tensor_copy(sumsL_bf[:, 0:512], pa[:])
                nc.vector.tensor_copy(sumsL_bf[:, 512:1024], pb[:])
                transpose_slots(s0, L, sumsL_bf, rdiag[:])
            linear_slots(s0, L, last=True)
            out_sb = sb.tile([B, OUT], f32)
            nc.any.tensor_copy(out_sb[:], psum_o[:])
            nc.scalar.dma_start(out_ext.ap(), out_sb[:])

    nc.compile()
    return nc


def _get_nc(early=True):
    if early not in _NCS:
        _NCS[early] = _build(early)
    return _NCS[early]


def _shard(values, coords, W, b):
    values = np.ascontiguousarray(values, dtype=np.float32)
    coords = np.ascontiguousarray(coords, dtype=np.float32)
    W = np.ascontiguousarray(W, dtype=np.float32)
    b = np.ascontiguousarray(b, dtype=np.float32)

    # bucketize exactly like the reference (same f32 comparisons)
    kx = (coords[:, 0:1] >= EDGES[None, :]).sum(1)
    ky = (coords[:, 1:2] >= EDGES[None, :]).sum(1)
    bins = (kx + 8 * ky).astype(np.int64)
    counts = np.bincount(bins, minlength=HW)        # real counts (pads excluded)
    order = np.argsort(bins, kind="stable")
    sbins = bins[order]

    # padded stream: each bin zero-padded to a multiple of FOLD so every
    # aligned quad is same-bin; sentinel tail (bin HW-1) fills to NTOT
    starts = np.searchsorted(sbins, np.arange(HW))
    ends = np.searchsorted(sbins, np.arange(HW), side="right")
    pos_list, bin_list = [], []
    for s in range(HW):
        n = ends[s] - starts[s]
        npad = (-n) % FOLD
        pos_list.append(order[starts[s]:ends[s]])
        pos_list.append(np.full(npad, -1, np.int64))
        bin_list.append(np.full(n + npad, s, np.int64))
    n1 = sum(len(p) for p in pos_list)
    assert n1 <= NTOT, f"padded stream {n1} exceeds capacity {NTOT}"
    pos_list.append(np.full(NTOT - n1, -1, np.int64))
    bin_list.append(np.full(NTOT - n1, HW - 1, np.int64))
    pos = np.concatenate(pos_list)                   # [NTOT], -1 = zero pad
    pbins = np.concatenate(bin_list)                 # [NTOT], non-decreasing
    qbins = pbins[0::FOLD]                           # per-quad bin (aligned)
    assert (pbins.reshape(-1, FOLD) == qbins[:, None]).all()

    # gather padded values [B, NTOT, C] in bf16 (zeros at sentinels)
    vstream = np.zeros((B, NTOT, C), dtype=BF16)
    real = pos >= 0
    vstream[:, real, :] = values[:, pos[real], :].astype(BF16)

    NQ = NS // FOLD                                  # quads per core (2080)
    early = True
    in_maps = []
    for i in range(N_CORES):
        qrun = qbins[i * NQ:(i + 1) * NQ]
        ubins = np.unique(qrun)
        assert len(ubins) <= L, f"core {i} spans {len(ubins)} bins > capacity {L}"
        qlocal = np.searchsorted(ubins, qrun).astype(np.float32)
        # slots 0..G0-1 must stop receiving contributions by unit FREEZE_FC
        sl = min(G0, len(ubins)) - 1
        if np.searchsorted(qrun, ubins[sl], "right") > FREEZE_FC * 128:
            early = False

        # binst[p, f<16] = slot of quad 128f+p; binst[p, 16] = slot of the
        # raw tail point 8192+p (quad 2048 + p//4)
        binst = np.empty((128, FC), np.float32)
        binst[:, :FC - 1] = qlocal[:2048].reshape(FC - 1, 128).T
        binst[:, FC - 1] = np.repeat(qlocal[2048:2080], FOLD)

        # device value layout: v[p, j=4f+r] = stream point 512f + 4p + r of
        # this core (so chunk r of unit f carries quad-element r); the raw
        # tail chunk j=64 carries points 8192+p directly
        v = vstream[:, i * NS:(i + 1) * NS, :]
        main = v[:, :2048 * FOLD, :].reshape(B, 16, 128, FOLD, C)
        main = main.transpose(2, 1, 3, 0, 4).reshape(128, 64, B, C)
        tailc = v[:, 2048 * FOLD:, :].transpose(1, 0, 2).reshape(128, 1, B, C)
        vdev = np.concatenate([main, tailc], axis=1).reshape(128, J * B * C)

        rec = np.zeros((L,), np.float32)
        rec[:len(ubins)] = 1.0 / np.maximum(counts[ubins], 1).astype(np.float32)
        wl = np.zeros((L * C, OUT), np.float32)
        for s, ub in enumerate(ubins):
            wl[s * C:(s + 1) * C] = W[ub * C:(ub + 1) * C]
        # pre-transpose so the device DMA is contiguous per partition:
        # wlt[p, kk*OUT + o] = wl[kk*128 + p, o]
        wlt = np.ascontiguousarray(
            wl.reshape(KK, 128, OUT).transpose(1, 0, 2)).reshape(128, KK * OUT)

        in_maps.append({
            "values": np.ascontiguousarray(vdev),
            "binst": np.ascontiguousarray(binst),
            "recdiag": np.ascontiguousarray(np.diag(rec)).astype(BF16),
            "W": wlt.astype(BF16),
        })
    return in_maps, early


def kernel(values, coords, W, b):
    in_maps, early = _shard(values, coords, W, b)
    nc = _get_nc(early)
    res = run_bass_kernel_spmd(nc, in_maps, core_ids=list(range(N_CORES)))
    parts = np.stack([np.asarray(res.results[i]["out"]) for i in range(N_CORES)])
    return parts.sum(axis=0, dtype=np.float32) + np.asarray(b, dtype=np.float32)
